# revision 4
# baseline (speedup 1.0000x reference)
"""Mixtral MoE layer (top-2 of 8 experts, SwiGLU) on 8 Trainium2 NeuronCores.

Strategy (expert-parallel, matches the sharding hint):
  - Host computes the tiny router (T x 8 logits, top-2, softmax weights).
  - Tokens are gathered per expert ("all-to-all"), padded to a fixed
    capacity C, and each of the 8 cores runs ONE expert's SwiGLU FFN over
    its token batch:   o = (silu(x@W1) * (x@W3)) @ W2,  rows scaled by the
    routing weight on-device.
  - Host scatter-adds the per-expert outputs back to the token layout.

Device kernel layout (per core):
  MM1/MM2: stationary = W1/W3 tile [d128, f128], moving = xT [d128, c]
           -> psum gT/uT [f128, c]; silu+mul -> aT [F, C] bf16 in SBUF.
  MM3:     stationary = aT tile [f128, c128], moving = W2 [f128, d]
           -> psum o [c128, d]; scaled by routing weight, DMA'd out.
All matmuls in bf16 with fp32 PSUM accumulation.
"""

import math

import numpy as np
import ml_dtypes

HIDDEN = 2048
FFN = 7168
N_EXPERTS = 8
TOP_K = 2
N_CORES = 8

KT = HIDDEN // 128   # 16 k-tiles over hidden dim
FT = FFN // 128      # 56 f-tiles over ffn dim
DCH = 512            # MM3 moving chunk over hidden dim
NDCH = HIDDEN // DCH

_NC_CACHE: dict = {}


def _cchunks(C: int) -> list[tuple[int, int]]:
    """Split C into balanced chunks of <=512 (each >=256 for sanity)."""
    n = (C + 511) // 512
    base, rem = divmod(C, n)
    out, off = [], 0
    for i in range(n):
        w = base + (1 if i < rem else 0)
        out.append((off, w))
        off += w
    return out


def _build_nc(C: int):
    import concourse.tile as tile
    from concourse import bacc, mybir

    BF = mybir.dt.bfloat16
    F32 = mybir.dt.float32
    NT = (C + 127) // 128
    chunks = _cchunks(C)

    nc = bacc.Bacc(
        "TRN2",
        target_bir_lowering=False,
        debug=False,
        enable_asserts=False,
        num_devices=N_CORES,
    )
    x_d = nc.dram_tensor("xT", [KT, 128, C], BF, kind="ExternalInput").ap()
    w1_d = nc.dram_tensor("w1p", [FT, 128, HIDDEN], BF, kind="ExternalInput").ap()
    w3_d = nc.dram_tensor("w3p", [FT, 128, HIDDEN], BF, kind="ExternalInput").ap()
    w2_d = nc.dram_tensor("w2", [FFN, HIDDEN], BF, kind="ExternalInput").ap()
    sc_d = nc.dram_tensor("scales", [128, NT], F32, kind="ExternalInput").ap()
    o_d = nc.dram_tensor("out", [C, HIDDEN], F32, kind="ExternalOutput").ap()

    x_r = x_d.rearrange("kt p c -> p kt c")
    w2_r = w2_d.rearrange("(ft p) d -> p ft d", p=128)

    with tile.TileContext(nc) as tc:
        with (
            tc.tile_pool(name="sc", bufs=1) as scpool,
            tc.tile_pool(name="aT", bufs=FT) as apool,
        ):
            sc = scpool.tile([128, NT], F32)
            nc.sync.dma_start(sc[:], sc_d)

            aT = []

            # ---- Phase 1: gT/uT = W1/W3^T @ x ; aT = silu(gT) * uT ----
            with (
                tc.tile_pool(name="xs", bufs=1) as xpool,
                tc.tile_pool(name="w13", bufs=3) as wpool,
                tc.tile_pool(name="ps1", bufs=2 if len(chunks) <= 2 else 1,
                             space="PSUM") as ps1,
                tc.tile_pool(name="silu", bufs=4) as spool,
            ):
                xs = xpool.tile([128, KT, C], BF)
                nc.sync.dma_start(xs[:], x_r)
                for ft in range(FT):
                    w1s = wpool.tile([128, HIDDEN], BF, tag="w1s")
                    nc.sync.dma_start(w1s[:], w1_d[ft])
                    w3s = wpool.tile([128, HIDDEN], BF, tag="w3s")
                    nc.sync.dma_start(w3s[:], w3_d[ft])

                    at = apool.tile([128, C], BF, tag="aT", name=f"aT{ft}")
                    aT.append(at)

                    gps = [ps1.tile([128, cw], F32, tag=f"g{ci}", name=f"g{ci}_{ft}")
                           for ci, (_, cw) in enumerate(chunks)]
                    ups = [ps1.tile([128, cw], F32, tag=f"u{ci}", name=f"u{ci}_{ft}")
                           for ci, (_, cw) in enumerate(chunks)]
                    for kt in range(KT):
                        lhs = w1s[:, kt * 128:(kt + 1) * 128]
                        for ci, (c0, cw) in enumerate(chunks):
                            nc.tensor.matmul(gps[ci][:], lhs, xs[:, kt, c0:c0 + cw],
                                             start=(kt == 0), stop=(kt == KT - 1))
                    for kt in range(KT):
                        lhs = w3s[:, kt * 128:(kt + 1) * 128]
                        for ci, (c0, cw) in enumerate(chunks):
                            nc.tensor.matmul(ups[ci][:], lhs, xs[:, kt, c0:c0 + cw],
                                             start=(kt == 0), stop=(kt == KT - 1))
                    for ci, (c0, cw) in enumerate(chunks):
                        st = spool.tile([128, cw], F32, tag=f"s{ci}", name=f"s{ci}_{ft}")
                        nc.scalar.activation(st[:], gps[ci][:],
                                             mybir.ActivationFunctionType.Silu)
                        nc.vector.tensor_mul(at[:, c0:c0 + cw], st[:], ups[ci][:])

            # ---- Phase 2: o = aT^T @ W2, scaled by routing weight ----
            with (
                tc.tile_pool(name="w2s", bufs=2) as w2pool,
                tc.tile_pool(name="ps2", bufs=4, space="PSUM") as ps2,
                tc.tile_pool(name="ost", bufs=4) as opool,
            ):
                for d in range(NDCH):
                    w2s = w2pool.tile([128, FT, DCH], BF)
                    nc.sync.dma_start(w2s[:], w2_r[:, :, d * DCH:(d + 1) * DCH])
                    for ct in range(NT):
                        c0 = ct * 128
                        cw = min(128, C - c0)
                        ops = ps2.tile([128, DCH], F32, tag="ops", name=f"ops_{d}_{ct}")
                        for ft in range(FT):
                            nc.tensor.matmul(ops[:cw], aT[ft][:, c0:c0 + cw],
                                             w2s[:, ft, :],
                                             start=(ft == 0), stop=(ft == FT - 1))
                        ot = opool.tile([128, DCH], F32, tag="ot", name=f"ot_{d}_{ct}")
                        nc.vector.tensor_scalar_mul(ot[:cw], ops[:cw], sc[:cw, ct:ct + 1])
                        nc.sync.dma_start(o_d[c0:c0 + cw, d * DCH:(d + 1) * DCH], ot[:cw])

    nc.compile()
    return nc


def _get_nc(C: int):
    if C not in _NC_CACHE:
        _NC_CACHE[C] = _build_nc(C)
    return _NC_CACHE[C]


def kernel(hidden_states, wg, w1, w3, w2):
    from concourse import bass_utils

    BF = ml_dtypes.bfloat16

    h = np.asarray(hidden_states, dtype=np.float32)
    orig_shape = h.shape
    h2 = np.ascontiguousarray(h.reshape(-1, HIDDEN))
    T = h2.shape[0]
    wg_ = np.asarray(wg, dtype=np.float32)

    # ---- Router (tiny): logits, top-2, softmax over the top-2 ----
    logits = h2 @ wg_.T  # [T, E] fp32
    t_idx = np.arange(T)
    i1 = np.argmax(logits, axis=1)
    v1 = logits[t_idx, i1]
    masked = logits.copy()
    masked[t_idx, i1] = -np.inf
    i2 = np.argmax(masked, axis=1)
    v2 = masked[t_idx, i2]
    ex2 = np.exp((v2 - v1).astype(np.float64))
    rw1 = (1.0 / (1.0 + ex2)).astype(np.float32)
    rw2 = (ex2 / (1.0 + ex2)).astype(np.float32)

    toks, wts = [], []
    for e in range(N_EXPERTS):
        m1 = i1 == e
        m2 = i2 == e
        toks.append(np.concatenate([t_idx[m1], t_idx[m2]]))
        wts.append(np.concatenate([rw1[m1], rw2[m2]]).astype(np.float32))
    maxcnt = max(len(t) for t in toks)
    C = max(512, 64 * math.ceil(maxcnt / 64))
    NT = (C + 127) // 128

    nc = _get_nc(C)

    # ---- Per-expert input prep ----
    w1b = np.asarray(w1, dtype=BF)
    w3b = np.asarray(w3, dtype=BF)
    w2b = np.ascontiguousarray(np.asarray(w2, dtype=BF))
    # [E, D, F] -> [E, FT, 128(d within kt), KT*128] tiled for contiguous DMA
    w1p = np.ascontiguousarray(
        w1b.reshape(N_EXPERTS, KT, 128, FT, 128).transpose(0, 3, 2, 1, 4)
    ).reshape(N_EXPERTS, FT, 128, HIDDEN)
    w3p = np.ascontiguousarray(
        w3b.reshape(N_EXPERTS, KT, 128, FT, 128).transpose(0, 3, 2, 1, 4)
    ).reshape(N_EXPERTS, FT, 128, HIDDEN)

    in_maps = []
    for e in range(N_EXPERTS):
        n = len(toks[e])
        xpad = np.zeros((C, HIDDEN), dtype=np.float32)
        xpad[:n] = h2[toks[e]]
        xT = np.ascontiguousarray(xpad.T.astype(BF)).reshape(KT, 128, C)
        sp = np.zeros((NT * 128,), dtype=np.float32)
        sp[:n] = wts[e]
        sc = np.ascontiguousarray(sp.reshape(NT, 128).T)
        in_maps.append({
            "xT": xT,
            "w1p": w1p[e],
            "w3p": w3p[e],
            "w2": w2b[e],
            "scales": sc,
        })

    res = bass_utils.run_bass_kernel_spmd(nc, in_maps, core_ids=list(range(N_CORES)))

    y = np.zeros((T, HIDDEN), dtype=np.float32)
    for e in range(N_EXPERTS):
        n = len(toks[e])
        if n:
            y[toks[e]] += res.results[e]["out"][:n]

    return y.reshape(orig_shape), logits


# revision 5
# speedup vs baseline: 76.9201x; 76.9201x over previous
"""Mixtral MoE layer (top-2 of 8 experts, SwiGLU) on 8 Trainium2 NeuronCores.

Strategy (expert-parallel, matches the sharding hint):
  - Host computes the tiny router (T x 8 logits, top-2, softmax weights).
  - Tokens are gathered per expert ("all-to-all"), padded to a fixed
    capacity C, and each of the 8 cores runs ONE expert's SwiGLU FFN over
    its token batch:   o = (silu(x@W1) * (x@W3)) @ W2,  rows scaled by the
    routing weight on-device.
  - Host scatter-adds the per-expert outputs back to the token layout.

Device kernel layout (per core):
  MM1/MM2: stationary = W1/W3 tile [d128, f128], moving = xT [d128, c]
           -> psum gT/uT [f128, c]; silu+mul -> aT [F, C] bf16 in SBUF.
  MM3:     stationary = aT tile [f128, c128], moving = W2 [f128, d]
           -> psum o [c128, d]; scaled by routing weight, DMA'd out.
All matmuls in bf16 with fp32 PSUM accumulation.
"""

import math

import numpy as np
import ml_dtypes

HIDDEN = 2048
FFN = 7168
N_EXPERTS = 8
TOP_K = 2
N_CORES = 8

KT = HIDDEN // 128   # 16 k-tiles over hidden dim
FT = FFN // 128      # 56 f-tiles over ffn dim
DCH = 512            # MM3 moving chunk over hidden dim
NDCH = HIDDEN // DCH

_NC_CACHE: dict = {}


def _cchunks(C: int) -> list[tuple[int, int]]:
    """Split C into balanced chunks of <=512 (each >=256 for sanity)."""
    n = (C + 511) // 512
    base, rem = divmod(C, n)
    out, off = [], 0
    for i in range(n):
        w = base + (1 if i < rem else 0)
        out.append((off, w))
        off += w
    return out


def _build_nc(C: int, repeat: int = 1):
    import concourse.tile as tile
    from concourse import bacc, mybir

    BF = mybir.dt.bfloat16
    F32 = mybir.dt.float32
    NT = (C + 127) // 128
    chunks = _cchunks(C)

    nc = bacc.Bacc(
        "TRN2",
        target_bir_lowering=False,
        debug=False,
        enable_asserts=False,
        num_devices=N_CORES,
    )
    x_d = nc.dram_tensor("xT", [KT, 128, C], BF, kind="ExternalInput").ap()
    w1_d = nc.dram_tensor("w1p", [FT, 128, HIDDEN], BF, kind="ExternalInput").ap()
    w3_d = nc.dram_tensor("w3p", [FT, 128, HIDDEN], BF, kind="ExternalInput").ap()
    w2_d = nc.dram_tensor("w2", [FFN, HIDDEN], BF, kind="ExternalInput").ap()
    sc_d = nc.dram_tensor("scales", [128, NT], F32, kind="ExternalInput").ap()
    o_d = nc.dram_tensor("out", [C, HIDDEN], F32, kind="ExternalOutput").ap()

    x_r = x_d.rearrange("kt p c -> p kt c")
    w2_r = w2_d.rearrange("(ft p) d -> p ft d", p=128)

    with tile.TileContext(nc) as tc:
        with tc.tile_pool(name="sc", bufs=1) as scpool:
            sc = scpool.tile([128, NT], F32)
            nc.sync.dma_start(sc[:], sc_d)

            for rep in range(repeat):
                _emit_body(nc, tc, tile, mybir, C, NT, chunks, rep,
                           x_r, w1_d, w3_d, w2_r, sc, o_d)

    nc.compile()
    return nc


def _emit_body(nc, tc, tile, mybir, C, NT, chunks, rep,
               x_r, w1_d, w3_d, w2_r, sc, o_d):
    BF = mybir.dt.bfloat16
    F32 = mybir.dt.float32

    with tc.tile_pool(name=f"aT{rep}", bufs=FT) as apool:
        aT = []
        # ---- Phase 1: gT/uT = W1/W3^T @ x ; aT = silu(gT) * uT ----
        with (
            tc.tile_pool(name=f"xs{rep}", bufs=1) as xpool,
            tc.tile_pool(name=f"w13{rep}", bufs=3) as wpool,
            tc.tile_pool(name=f"ps1{rep}", bufs=2 if len(chunks) <= 2 else 1,
                         space="PSUM") as ps1,
            tc.tile_pool(name=f"silu{rep}", bufs=4) as spool,
        ):
            xs = xpool.tile([128, KT, C], BF, name=f"xs{rep}")
            nc.sync.dma_start(xs[:], x_r)
            for ft in range(FT):
                w1s = wpool.tile([128, HIDDEN], BF, tag="w1s", name=f"w1s_{rep}_{ft}")
                nc.sync.dma_start(w1s[:], w1_d[ft])
                w3s = wpool.tile([128, HIDDEN], BF, tag="w3s", name=f"w3s_{rep}_{ft}")
                nc.sync.dma_start(w3s[:], w3_d[ft])

                at = apool.tile([128, C], BF, tag="aT", name=f"aT_{rep}_{ft}")
                aT.append(at)

                gps = [ps1.tile([128, cw], F32, tag=f"g{ci}", name=f"g{ci}_{rep}_{ft}")
                       for ci, (_, cw) in enumerate(chunks)]
                ups = [ps1.tile([128, cw], F32, tag=f"u{ci}", name=f"u{ci}_{rep}_{ft}")
                       for ci, (_, cw) in enumerate(chunks)]
                for kt in range(KT):
                    lhs = w1s[:, kt * 128:(kt + 1) * 128]
                    for ci, (c0, cw) in enumerate(chunks):
                        nc.tensor.matmul(gps[ci][:], lhs, xs[:, kt, c0:c0 + cw],
                                         start=(kt == 0), stop=(kt == KT - 1))
                for kt in range(KT):
                    lhs = w3s[:, kt * 128:(kt + 1) * 128]
                    for ci, (c0, cw) in enumerate(chunks):
                        nc.tensor.matmul(ups[ci][:], lhs, xs[:, kt, c0:c0 + cw],
                                         start=(kt == 0), stop=(kt == KT - 1))
                for ci, (c0, cw) in enumerate(chunks):
                    st = spool.tile([128, cw], F32, tag=f"s{ci}",
                                    name=f"s{ci}_{rep}_{ft}")
                    nc.scalar.activation(st[:], gps[ci][:],
                                         mybir.ActivationFunctionType.Silu)
                    nc.vector.tensor_mul(at[:, c0:c0 + cw], st[:], ups[ci][:])

        # ---- Phase 2: o = aT^T @ W2, scaled by routing weight ----
        with (
            tc.tile_pool(name=f"w2s{rep}", bufs=2) as w2pool,
            tc.tile_pool(name=f"ps2{rep}", bufs=4, space="PSUM") as ps2,
            tc.tile_pool(name=f"ost{rep}", bufs=4) as opool,
        ):
            for d in range(NDCH):
                w2s = w2pool.tile([128, FT, DCH], BF, tag="w2s", name=f"w2s_{rep}_{d}")
                nc.sync.dma_start(w2s[:], w2_r[:, :, d * DCH:(d + 1) * DCH])
                for ct in range(NT):
                    c0 = ct * 128
                    cw = min(128, C - c0)
                    ops = ps2.tile([128, DCH], F32, tag="ops", name=f"ops_{rep}_{d}_{ct}")
                    for ft in range(FT):
                        nc.tensor.matmul(ops[:cw], aT[ft][:, c0:c0 + cw],
                                         w2s[:, ft, :],
                                         start=(ft == 0), stop=(ft == FT - 1))
                    ot = opool.tile([128, DCH], F32, tag="ot", name=f"ot_{rep}_{d}_{ct}")
                    nc.vector.tensor_scalar_mul(ot[:cw], ops[:cw], sc[:cw, ct:ct + 1])
                    nc.sync.dma_start(o_d[c0:c0 + cw, d * DCH:(d + 1) * DCH], ot[:cw])


def _get_nc(C: int):
    if C not in _NC_CACHE:
        _NC_CACHE[C] = _build_nc(C)
    return _NC_CACHE[C]


def kernel(hidden_states, wg, w1, w3, w2):
    from concourse import bass_utils

    BF = ml_dtypes.bfloat16

    h = np.asarray(hidden_states, dtype=np.float32)
    orig_shape = h.shape
    h2 = np.ascontiguousarray(h.reshape(-1, HIDDEN))
    T = h2.shape[0]
    wg_ = np.asarray(wg, dtype=np.float32)

    # ---- Router (tiny): logits, top-2, softmax over the top-2 ----
    logits = h2 @ wg_.T  # [T, E] fp32
    t_idx = np.arange(T)
    i1 = np.argmax(logits, axis=1)
    v1 = logits[t_idx, i1]
    masked = logits.copy()
    masked[t_idx, i1] = -np.inf
    i2 = np.argmax(masked, axis=1)
    v2 = masked[t_idx, i2]
    ex2 = np.exp((v2 - v1).astype(np.float64))
    rw1 = (1.0 / (1.0 + ex2)).astype(np.float32)
    rw2 = (ex2 / (1.0 + ex2)).astype(np.float32)

    toks, wts = [], []
    for e in range(N_EXPERTS):
        m1 = i1 == e
        m2 = i2 == e
        toks.append(np.concatenate([t_idx[m1], t_idx[m2]]))
        wts.append(np.concatenate([rw1[m1], rw2[m2]]).astype(np.float32))
    maxcnt = max(len(t) for t in toks)
    C = max(512, 64 * math.ceil(maxcnt / 64))
    NT = (C + 127) // 128

    nc = _get_nc(C)

    # ---- Per-expert input prep ----
    w1b = np.asarray(w1, dtype=BF)
    w3b = np.asarray(w3, dtype=BF)
    w2b = np.ascontiguousarray(np.asarray(w2, dtype=BF))
    # [E, D, F] -> [E, FT, 128(d within kt), KT*128] tiled for contiguous DMA
    w1p = np.ascontiguousarray(
        w1b.reshape(N_EXPERTS, KT, 128, FT, 128).transpose(0, 3, 2, 1, 4)
    ).reshape(N_EXPERTS, FT, 128, HIDDEN)
    w3p = np.ascontiguousarray(
        w3b.reshape(N_EXPERTS, KT, 128, FT, 128).transpose(0, 3, 2, 1, 4)
    ).reshape(N_EXPERTS, FT, 128, HIDDEN)

    in_maps = []
    for e in range(N_EXPERTS):
        n = len(toks[e])
        xpad = np.zeros((C, HIDDEN), dtype=np.float32)
        xpad[:n] = h2[toks[e]]
        xT = np.ascontiguousarray(xpad.T.astype(BF)).reshape(KT, 128, C)
        sp = np.zeros((NT * 128,), dtype=np.float32)
        sp[:n] = wts[e]
        sc = np.ascontiguousarray(sp.reshape(NT, 128).T)
        in_maps.append({
            "xT": xT,
            "w1p": w1p[e],
            "w3p": w3p[e],
            "w2": w2b[e],
            "scales": sc,
        })

    res = bass_utils.run_bass_kernel_spmd(nc, in_maps, core_ids=list(range(N_CORES)))

    y = np.zeros((T, HIDDEN), dtype=np.float32)
    for e in range(N_EXPERTS):
        n = len(toks[e])
        if n:
            y[toks[e]] += res.results[e]["out"][:n]

    return y.reshape(orig_shape), logits


# revision 7
# speedup vs baseline: 93.1659x; 1.2112x over previous
"""Mixtral MoE layer (top-2 of 8 experts, SwiGLU) on 8 Trainium2 NeuronCores.

Strategy (expert-parallel, matches the sharding hint):
  - Host computes the tiny router (T x 8 logits, top-2, softmax weights).
  - Tokens are gathered per expert ("all-to-all"), padded to a fixed
    capacity C, and each of the 8 cores runs ONE expert's SwiGLU FFN over
    its token batch:   o = (silu(x@W1) * (x@W3)) @ W2,  rows scaled by the
    routing weight on-device.
  - Host scatter-adds the per-expert outputs back to the token layout.

Device kernel layout (per core):
  MM1/MM2: stationary = W1/W3 tile [d128, f128], moving = xT [d128, c]
           -> psum gT/uT [f128, c]; silu+mul -> aT [F, C] bf16 in SBUF.
  MM3:     stationary = aT tile [f128, c128], moving = W2 [f128, d]
           -> psum o [c128, d]; scaled by routing weight, DMA'd out.
All matmuls in bf16 with fp32 PSUM accumulation.
"""

import math

import numpy as np
import ml_dtypes

HIDDEN = 2048
FFN = 7168
N_EXPERTS = 8
TOP_K = 2
N_CORES = 8

KT = HIDDEN // 128   # 16 k-tiles over hidden dim
FT = FFN // 128      # 56 f-tiles over ffn dim
DCH = 512            # MM3 moving chunk over hidden dim
NDCH = HIDDEN // DCH

_NC_CACHE: dict = {}


def _cchunks(C: int) -> list[tuple[int, int]]:
    """Split C into balanced chunks of <=512 (each >=256 for sanity)."""
    n = (C + 511) // 512
    base, rem = divmod(C, n)
    out, off = [], 0
    for i in range(n):
        w = base + (1 if i < rem else 0)
        out.append((off, w))
        off += w
    return out


def _build_nc(C: int, repeat: int = 1):
    import concourse.tile as tile
    from concourse import bacc, mybir

    BF = mybir.dt.bfloat16
    F32 = mybir.dt.float32
    NT = (C + 127) // 128
    chunks = _cchunks(C)

    nc = bacc.Bacc(
        "TRN2",
        target_bir_lowering=False,
        debug=False,
        enable_asserts=False,
        num_devices=N_CORES,
    )
    x_d = nc.dram_tensor("xT", [KT, 128, C], BF, kind="ExternalInput").ap()
    w1_d = nc.dram_tensor("w1p", [FT, 128, HIDDEN], BF, kind="ExternalInput").ap()
    w3_d = nc.dram_tensor("w3p", [FT, 128, HIDDEN], BF, kind="ExternalInput").ap()
    w2_d = nc.dram_tensor("w2", [FFN, HIDDEN], BF, kind="ExternalInput").ap()
    sc_d = nc.dram_tensor("scales", [128, NT], F32, kind="ExternalInput").ap()
    o_d = nc.dram_tensor("out", [C, HIDDEN], F32, kind="ExternalOutput").ap()

    x_r = x_d.rearrange("kt p c -> p kt c")
    w2_r = w2_d.rearrange("(ft p) d -> p ft d", p=128)

    with tile.TileContext(nc) as tc:
        with tc.tile_pool(name="sc", bufs=1) as scpool:
            sc = scpool.tile([128, NT], F32)
            nc.sync.dma_start(sc[:], sc_d)

            for rep in range(repeat):
                _emit_body(nc, tc, tile, mybir, C, NT, chunks, rep,
                           x_r, w1_d, w3_d, w2_r, sc, o_d)

    nc.compile()
    return nc


def _emit_body(nc, tc, tile, mybir, C, NT, chunks, rep,
               x_r, w1_d, w3_d, w2_r, sc, o_d):
    BF = mybir.dt.bfloat16
    F32 = mybir.dt.float32

    FQ = FT // 4  # w2 tiles arrive in quarters of the f-range

    with (
        tc.tile_pool(name=f"aT{rep}", bufs=FT) as apool,
        tc.tile_pool(name=f"w2s{rep}", bufs=6) as w2pool,
    ):
        aT = []
        w2tiles = [[None] * 4 for _ in range(NDCH)]

        def emit_w2_dma(d, q):
            w2s = w2pool.tile([128, FQ, DCH], BF, tag="w2s", name=f"w2s_{rep}_{d}_{q}")
            nc.sync.dma_start(
                w2s[:], w2_r[:, q * FQ:(q + 1) * FQ, d * DCH:(d + 1) * DCH])
            w2tiles[d][q] = w2s

        # ---- Phase 1: gT/uT = W1/W3^T @ x ; aT = silu(gT) * uT ----
        with (
            tc.tile_pool(name=f"xs{rep}", bufs=1) as xpool,
            tc.tile_pool(name=f"w13{rep}", bufs=4) as wpool,
            tc.tile_pool(name=f"ps1{rep}", bufs=2 if len(chunks) <= 2 else 1,
                         space="PSUM") as ps1,
            tc.tile_pool(name=f"silu{rep}", bufs=4) as spool,
        ):
            xs = xpool.tile([128, KT, C], BF, name=f"xs{rep}")
            nc.sync.dma_start(xs[:], x_r)
            for ft in range(FT):
                if ft >= FT - 4:
                    emit_w2_dma(0, ft - (FT - 4))
                w1s = wpool.tile([128, HIDDEN], BF, tag="w1s", name=f"w1s_{rep}_{ft}")
                nc.sync.dma_start(w1s[:], w1_d[ft])
                w3s = wpool.tile([128, HIDDEN], BF, tag="w3s", name=f"w3s_{rep}_{ft}")
                nc.sync.dma_start(w3s[:], w3_d[ft])

                at = apool.tile([128, C], BF, tag="aT", name=f"aT_{rep}_{ft}")
                aT.append(at)

                gps = [ps1.tile([128, cw], F32, tag=f"g{ci}", name=f"g{ci}_{rep}_{ft}")
                       for ci, (_, cw) in enumerate(chunks)]
                ups = [ps1.tile([128, cw], F32, tag=f"u{ci}", name=f"u{ci}_{rep}_{ft}")
                       for ci, (_, cw) in enumerate(chunks)]
                for kt in range(KT):
                    lhs = w1s[:, kt * 128:(kt + 1) * 128]
                    for ci, (c0, cw) in enumerate(chunks):
                        nc.tensor.matmul(gps[ci][:], lhs, xs[:, kt, c0:c0 + cw],
                                         start=(kt == 0), stop=(kt == KT - 1))
                for kt in range(KT):
                    lhs = w3s[:, kt * 128:(kt + 1) * 128]
                    for ci, (c0, cw) in enumerate(chunks):
                        nc.tensor.matmul(ups[ci][:], lhs, xs[:, kt, c0:c0 + cw],
                                         start=(kt == 0), stop=(kt == KT - 1))
                for ci, (c0, cw) in enumerate(chunks):
                    st = spool.tile([128, cw], F32, tag=f"s{ci}",
                                    name=f"s{ci}_{rep}_{ft}")
                    nc.scalar.activation(st[:], gps[ci][:],
                                         mybir.ActivationFunctionType.Silu)
                    nc.vector.tensor_mul(at[:, c0:c0 + cw], st[:], ups[ci][:])

        # ---- Phase 2: o = aT^T @ W2, scaled by routing weight ----
        with (
            tc.tile_pool(name=f"ps2{rep}", bufs=4, space="PSUM") as ps2,
            tc.tile_pool(name=f"ost{rep}", bufs=4) as opool,
        ):
            for d in range(NDCH):
                for q in range(4):
                    if w2tiles[d][q] is None:
                        emit_w2_dma(d, q)
                for ct in range(NT):
                    c0 = ct * 128
                    cw = min(128, C - c0)
                    ops = ps2.tile([128, DCH], F32, tag="ops", name=f"ops_{rep}_{d}_{ct}")
                    for ft in range(FT):
                        nc.tensor.matmul(ops[:cw], aT[ft][:, c0:c0 + cw],
                                         w2tiles[d][ft // FQ][:, ft % FQ, :],
                                         start=(ft == 0), stop=(ft == FT - 1))
                    ot = opool.tile([128, DCH], F32, tag="ot", name=f"ot_{rep}_{d}_{ct}")
                    nc.vector.tensor_scalar_mul(ot[:cw], ops[:cw], sc[:cw, ct:ct + 1])
                    nc.sync.dma_start(o_d[c0:c0 + cw, d * DCH:(d + 1) * DCH], ot[:cw])


def _get_nc(C: int):
    if C not in _NC_CACHE:
        _NC_CACHE[C] = _build_nc(C)
    return _NC_CACHE[C]


def kernel(hidden_states, wg, w1, w3, w2):
    from concourse import bass_utils

    BF = ml_dtypes.bfloat16

    h = np.asarray(hidden_states, dtype=np.float32)
    orig_shape = h.shape
    h2 = np.ascontiguousarray(h.reshape(-1, HIDDEN))
    T = h2.shape[0]
    wg_ = np.asarray(wg, dtype=np.float32)

    # ---- Router (tiny): logits, top-2, softmax over the top-2 ----
    logits = h2 @ wg_.T  # [T, E] fp32
    t_idx = np.arange(T)
    i1 = np.argmax(logits, axis=1)
    v1 = logits[t_idx, i1]
    masked = logits.copy()
    masked[t_idx, i1] = -np.inf
    i2 = np.argmax(masked, axis=1)
    v2 = masked[t_idx, i2]
    ex2 = np.exp((v2 - v1).astype(np.float64))
    rw1 = (1.0 / (1.0 + ex2)).astype(np.float32)
    rw2 = (ex2 / (1.0 + ex2)).astype(np.float32)

    toks, wts = [], []
    for e in range(N_EXPERTS):
        m1 = i1 == e
        m2 = i2 == e
        toks.append(np.concatenate([t_idx[m1], t_idx[m2]]))
        wts.append(np.concatenate([rw1[m1], rw2[m2]]).astype(np.float32))
    maxcnt = max(len(t) for t in toks)
    C = max(512, 8 * math.ceil(maxcnt / 8))
    NT = (C + 127) // 128

    nc = _get_nc(C)

    # ---- Per-expert input prep ----
    w1b = np.asarray(w1, dtype=BF)
    w3b = np.asarray(w3, dtype=BF)
    w2b = np.ascontiguousarray(np.asarray(w2, dtype=BF))
    # [E, D, F] -> [E, FT, 128(d within kt), KT*128] tiled for contiguous DMA
    w1p = np.ascontiguousarray(
        w1b.reshape(N_EXPERTS, KT, 128, FT, 128).transpose(0, 3, 2, 1, 4)
    ).reshape(N_EXPERTS, FT, 128, HIDDEN)
    w3p = np.ascontiguousarray(
        w3b.reshape(N_EXPERTS, KT, 128, FT, 128).transpose(0, 3, 2, 1, 4)
    ).reshape(N_EXPERTS, FT, 128, HIDDEN)

    in_maps = []
    for e in range(N_EXPERTS):
        n = len(toks[e])
        xpad = np.zeros((C, HIDDEN), dtype=np.float32)
        xpad[:n] = h2[toks[e]]
        xT = np.ascontiguousarray(xpad.T.astype(BF)).reshape(KT, 128, C)
        sp = np.zeros((NT * 128,), dtype=np.float32)
        sp[:n] = wts[e]
        sc = np.ascontiguousarray(sp.reshape(NT, 128).T)
        in_maps.append({
            "xT": xT,
            "w1p": w1p[e],
            "w3p": w3p[e],
            "w2": w2b[e],
            "scales": sc,
        })

    res = bass_utils.run_bass_kernel_spmd(nc, in_maps, core_ids=list(range(N_CORES)))

    y = np.zeros((T, HIDDEN), dtype=np.float32)
    for e in range(N_EXPERTS):
        n = len(toks[e])
        if n:
            y[toks[e]] += res.results[e]["out"][:n]

    return y.reshape(orig_shape), logits
